# revision 34
# baseline (speedup 1.0000x reference)
"""Trainium2 Bass kernel for DilatedReparamConv (5-branch depthwise conv + BN + SiLU + identity BN).

out = BN_id(x) + sum_i silu(BN_i(dwconv_i(x)))   for branches
      (5,d1), (7,d2), (3,d3), (3,d4), (3,d5), all SAME padding.

Strategy (8 NeuronCores, SPMD):
  - Shard the 256 channels across 8 cores (32 ch/core, all 32 images).
  - Depthwise conv on TensorE in fp8 (e4m3) DoubleRowSwInterleave: per
    channel-pair, a block-diagonal banded Toeplitz matrix contracts the kh
    tap-stack in one matmul; kw taps are free-dim shifts into a W-padded x
    tile. DR mode runs 2 slots/cycle (0.5 cyc/col): the slot pair carries
    either (x8, resid) error feedback [EF] or two packed taps (x8@dxA,
    x8@dxB) [PACK], per the MM_PLAN below.
  - BN+SiLU fused into ScalarE PSUM->SBUF eviction (per-partition scale/bias
    APs), one [128, 2048] activation per branch (4 PSUM banks).
  - fp16 branch tiles; DVE tree-sum in fp16 (2x_1p mode); identity branch
    fused via affine_then_add custom DVE op. fp16 output.
"""

import sys

sys.path.insert(0, "/opt/trn_rl_repo")

import numpy as np
import ml_dtypes

import concourse.bass as bass
import concourse.mybir as mybir
from concourse import bacc, tile
from concourse.bass_utils import run_bass_kernel_spmd

# ---------------------------------------------------------------- problem dims
B, C, H, W = 32, 256, 64, 64
EPS = 1e-5
BRANCH_CFG = [(5, 1), (7, 2), (3, 3), (3, 4), (3, 5)]  # (kernel, dilation)
N_CORES = 8
C_CORE = C // N_CORES          # 32 channels per core
PAIRS = C_CORE // 2            # 16 channel-pairs per core
PAD = 6                        # max dilation*(ks-1)//2 across branches
WP = W + 2 * PAD               # padded width = 76

F8 = ml_dtypes.float8_e4m3
F16 = np.float16

W_CHUNK, N_CHUNKS = 16, 4      # 16 w-cols * B = 512 psum cols per chunk

# Which kw taps of each branch get error feedback (the rest are packed
# two-taps-per-matmul). "all" = full EF.
# P13-ctr: EF only the center kw tap of each branch (13 MMs total).
EF_SETS = [{2}, {3}, {1}, {1}, {1}]


def _branch_taps(br):
    ks, dil = BRANCH_CFG[br]
    pad = dil * (ks - 1) // 2
    return [(j, dil * j - pad) for j in range(ks)]


def _mm_plan():
    """Per branch: list of MMs. Each MM = (slotA, slotB), slot =
    (plane, dx, tap_j, half) with plane in {0:x8, 1:resid}; half -> use
    T8/2 matrix. Slots are ordered so that byte offset(A) < offset(B)."""
    plan = []
    for br in range(5):
        taps = _branch_taps(br)
        ef = EF_SETS[br]
        if ef is None:
            ef = {j for j, _ in taps}
        mms = []
        packed = [t for t in taps if t[0] not in ef]
        for j, dx in taps:
            if j in ef:
                # (x8@dx, resid@dx): offsets x8 < resid plane -> ordered
                mms.append(((0, dx, j, False), (1, dx, j, True)))
        for i in range(0, len(packed) - 1, 2):
            (ja, dxa), (jb, dxb) = packed[i], packed[i + 1]
            mms.append(((0, dxa, ja, False), (0, dxb, jb, False)))
        if len(packed) % 2:
            j, dx = packed[-1]
            # lone tap: EF it (resid slot is free anyway)
            mms.append(((0, dx, j, False), (1, dx, j, True)))
        plan.append(mms)
    return plan


MM_PLAN = _mm_plan()
N_MMS = sum(len(m) for m in MM_PLAN)

# packed per-partition byte layout of the xw tile (all uint8 columns)
# x8/resid are element-interleaved ([WP*B, 2] pairs); the identity branch is
# reconstructed on DVE as s*(x8 + resid/2) + b, so no natural-x plane.
X8R_OFF = 0                    # (x8, resid) pairs [WP*B, 2] fp8
W8_OFF = 2 * WP * B            # weights [N_MMS, 2, 128] fp8 interleaved
XW_COLS = W8_OFF + N_MMS * 2 * 128

_CACHE: dict = {}


# =====================================================================
# device build
# =====================================================================
def build_nc():
    nc = bacc.Bacc("TRN2", target_bir_lowering=False, debug=False, num_devices=N_CORES)
    f32 = mybir.dt.float32
    f16 = mybir.dt.float16
    u8 = mybir.dt.uint8
    f8 = mybir.dt.float8e4

    xw = nc.dram_tensor("xw", [PAIRS, 128, XW_COLS], u8, kind="ExternalInput").ap()
    scbi = nc.dram_tensor("scbi", [128, 2 * PAIRS * 8], f32, kind="ExternalInput").ap()
    yt = nc.dram_tensor("yt", [PAIRS, 128, B * W], f16, kind="ExternalOutput").ap()

    NBW = B * W  # 2048

    with tile.TileContext(nc) as tc:
        with (
            tc.tile_pool(name="consts", bufs=1) as consts,
            tc.tile_pool(name="xwp", bufs=6) as xwp,
            tc.tile_pool(name="accp", bufs=4) as accp,
            tc.tile_pool(name="tp", bufs=17) as tp,
            tc.tile_pool(name="tsum", bufs=6) as tsum,
            tc.tile_pool(name="psum", bufs=2, space="PSUM") as psum,
        ):
            scbi_t = consts.tile([128, 2 * PAIRS * 8], f32)
            nc.sync.dma_start(out=scbi_t[:], in_=scbi)
            sc_t = scbi_t[:, : PAIRS * 8]
            bi_t = scbi_t[:, PAIRS * 8 :]

            wf8 = scbi_t.bitcast(f8)

            def load_pair(p):
                xw_t = xwp.tile([128, XW_COLS], u8)
                # weights first (first matmul needs them), then the plane in
                # two pieces so early chunks can start before the tail lands
                nc.sync.dma_start(out=xw_t[:, W8_OFF:], in_=xw[p][:, W8_OFF:])
                q = W8_OFF // 2
                for i in range(2):
                    nc.sync.dma_start(
                        out=xw_t[:, i * q : (i + 1) * q],
                        in_=xw[p][:, i * q : (i + 1) * q],
                    )
                return xw_t

            def mm_branch(xw_t, p, br, warm=False):
                xw_f8 = xw_t.bitcast(f8)

                def rhs_ap(mm, cch):
                    # byte offset of slot s in the interleaved (x8, resid)
                    # plane: 2*element + plane
                    (pa, dxa, _, _), (pb, dxb, _, _) = mm
                    offa = 2 * (cch * W_CHUNK + PAD + dxa) * B + pa
                    offb = 2 * (cch * W_CHUNK + PAD + dxb) * B + pb
                    assert offb > offa
                    r = xw_f8[:, offa : offa + 2 * W_CHUNK * B].unsqueeze(1)
                    ap = r.ap
                    ap[1] = [offb - offa, 2]   # slot dim
                    ap[2] = [2, W_CHUNK * B]   # (w, b) columns, every other byte
                    r.ap = ap
                    return r

                mms = MM_PLAN[br]
                ps = psum.tile([128, NBW], f32)
                if warm:
                    # warm up the PE clock (HAM) during the initial DMA with
                    # dummy matmuls on the tiny already-loaded scbi bytes;
                    # the first real start=True matmul clears the garbage
                    for _ in range(16):
                        nc.tensor.matmul(
                            ps[:, :256],
                            wf8[:, :256],
                            wf8[:, :512].rearrange("p (n two) -> p two n", two=2),
                            start=True,
                            stop=True,
                            perf_mode=mybir.MatmulPerfMode.DoubleRowSwInterleave,
                        )
                # mi-outer: 4 consecutive matmuls share one weight load
                for mi, mm in enumerate(mms):
                    lhsT = xw_f8[
                        :,
                        W8_OFF + (_MM_BASE[br] + mi) * 256 :
                        W8_OFF + (_MM_BASE[br] + mi + 1) * 256,
                    ]
                    for cch in range(N_CHUNKS):
                        nc.tensor.matmul(
                            ps[:, cch * 512 : (cch + 1) * 512],
                            lhsT,
                            rhs_ap(mm, cch),
                            start=(mi == 0),
                            stop=(mi == len(mms) - 1),
                            perf_mode=mybir.MatmulPerfMode.DoubleRowSwInterleave,
                        )
                t = tp.tile([128, NBW], f16)
                halves = 1
                for h in range(halves):
                    lo = h * NBW // halves
                    hi = (h + 1) * NBW // halves
                    nc.scalar.activation(
                        t[:, lo:hi],
                        ps[:, lo:hi],
                        mybir.ActivationFunctionType.Silu,
                        bias=bi_t[:, p * 8 + br : p * 8 + br + 1],
                        scale=sc_t[:, p * 8 + br : p * 8 + br + 1],
                    )
                return t

            def finish_pair(xw_t, p, ts):
                xw_f8 = xw_t.bitcast(f8)

                def id_view(plane):
                    # x8 (plane 0) or resid (plane 1) at dx=0, [128, 2048]
                    s = 2 * PAD * B + plane
                    r = xw_f8[:, s : s + 2 * W * B]
                    ap = r.ap
                    ap[1] = [2, W * B]
                    r.ap = ap
                    return r

                # acc = s_id*(x8 + resid/2) + b_id + sum(t); tree-sum
                acc = accp.tile([128, NBW], f16)
                nc.vector.affine_then_add(
                    acc[:],
                    id_view(0),
                    ts[0][:],
                    sc_t[:, p * 8 + 5 : p * 8 + 6],
                    bi_t[:, p * 8 + 5 : p * 8 + 6],
                )
                nc.vector.affine_then_add(
                    acc[:],
                    id_view(1),
                    acc[:],
                    sc_t[:, p * 8 + 6 : p * 8 + 7],
                    bi_t[:, p * 8 + 6 : p * 8 + 7],
                )
                t12 = tsum.tile([128, NBW], f16)
                nc.vector.tensor_tensor(t12[:], ts[1][:], ts[2][:], op=mybir.AluOpType.add)
                nc.vector.tensor_tensor(acc[:], acc[:], t12[:], op=mybir.AluOpType.add)
                t34 = tsum.tile([128, NBW], f16)
                nc.vector.tensor_tensor(t34[:], ts[3][:], ts[4][:], op=mybir.AluOpType.add)
                nc.vector.tensor_tensor(acc[:], acc[:], t34[:], op=mybir.AluOpType.add)

                nc.sync.dma_start(out=yt[p][:, : NBW // 2], in_=acc[:, : NBW // 2])
                nc.sync.dma_start(out=yt[p][:, NBW // 2 :], in_=acc[:, NBW // 2 :])

            # three pairs in flight: each psum-tile reuse has two other
            # pairs' matmul blocks to cover the eviction latency
            groups = [[0, 1, 2], [3, 4, 5], [6, 7, 8], [9, 10, 11],
                      [12, 13], [14, 15]]
            for g in groups:
                xs = [load_pair(p) for p in g]
                tss = [[] for _ in g]
                for br in range(5):
                    for k, p in enumerate(g):
                        tss[k].append(
                            mm_branch(xs[k], p, br, warm=(br == 0 and p == 0))
                        )
                for k, p in enumerate(g):
                    finish_pair(xs[k], p, tss[k])

    nc.compile()
    return nc


_MM_BASE = [0]
for _m in MM_PLAN:
    _MM_BASE.append(_MM_BASE[-1] + len(_m))


# ------------------------------------------------------------------ host prep
def _toeplitz_taps():
    """T[c, br][j] = [H, H] banded matrix (hi, ho) for kw tap j of branch br,
    returned as one [C, TOT_TAPS, H, H] array with per-branch tap offsets."""
    tot = sum(ks for ks, _ in BRANCH_CFG)
    base = [0]
    for ks, _ in BRANCH_CFG:
        base.append(base[-1] + ks)
    return tot, base


def _host_prep(x, id_bn, w5, w7, w3a, w3b, w3c, bn_gamma, bn_beta, bn_mean, bn_var):
    x = np.asarray(x, np.float32)
    weights = [np.asarray(w, np.float32) for w in (w5, w7, w3a, w3b, w3c)]
    id_bn = np.asarray(id_bn, np.float32)
    bn_gamma = np.asarray(bn_gamma, np.float32)
    bn_beta = np.asarray(bn_beta, np.float32)
    bn_mean = np.asarray(bn_mean, np.float32)
    bn_var = np.asarray(bn_var, np.float32)

    # per-(branch, channel) gain so T8 uses the e4m3 range well
    gain = np.empty((5, C), np.float32)
    for br in range(5):
        wmax = np.abs(weights[br][:, 0]).max(axis=(1, 2))
        gain[br] = 8.0 / np.maximum(wmax, 1e-8)

    # scale/bias tables: branches (scale / gain), identity, identity/2, zero
    S = np.zeros((8, C), np.float32)
    Bv = np.zeros((8, C), np.float32)
    for i in range(5):
        s = bn_gamma[i] / np.sqrt(bn_var[i] + EPS)
        S[i] = s / gain[i]
        Bv[i] = bn_beta[i] - bn_mean[i] * s
    s = id_bn[0] / np.sqrt(id_bn[3] + EPS)
    S[5] = s
    Bv[5] = id_bn[1] - id_bn[2] * s
    S[6] = s / 2.0

    # scbi per core: [128, PAIRS*8] scale | [128, PAIRS*8] bias
    # partition layout (ci*H + h), column layout p*8 + i
    Sr = S.reshape(8, N_CORES, PAIRS, 2)     # [8, k, p, ci]
    Br = Bv.reshape(8, N_CORES, PAIRS, 2)
    scbi_all = np.empty((N_CORES, 128, 2 * PAIRS * 8), np.float32)
    for ci in range(2):
        blk = slice(ci * H, (ci + 1) * H)
        scbi_all[:, blk, : PAIRS * 8] = np.transpose(
            Sr[:, :, :, ci], (1, 2, 0)
        ).reshape(N_CORES, 1, PAIRS * 8)
        scbi_all[:, blk, PAIRS * 8 :] = np.transpose(
            Br[:, :, :, ci], (1, 2, 0)
        ).reshape(N_CORES, 1, PAIRS * 8)

    # x planes, w-major free layout: [C, H, WP, B]
    xt = np.transpose(x, (1, 2, 3, 0))                 # [C, H, W, B]
    xp = np.zeros((C, H, WP, B), np.float32)
    xp[:, :, PAD : PAD + W, :] = xt
    x8 = xp.astype(F8)
    r8 = (2.0 * (xp - x8.astype(np.float32))).astype(F8)

    # per-tap banded Toeplitz matrices, with gain folded in
    # T8[c, br, j][hi, ho], T8q = e4m3(T8/2)
    T8_list = []
    T8q_list = []
    for br, (ks, dil) in enumerate(BRANCH_CFG):
        pad = dil * (ks - 1) // 2
        wbr = weights[br][:, 0] * gain[br][:, None, None]   # [C, ks, ks]
        Tb = np.zeros((C, ks, H, H), np.float32)
        for kw in range(ks):
            for kh in range(ks):
                off = dil * kh - pad
                ho = np.arange(max(0, -off), min(H, H - off))
                Tb[:, kw, ho + off, ho] = wbr[:, kh, kw][:, None]
        T8 = Tb.astype(F8)
        T8q = (T8.astype(np.float32) / 2.0).astype(F8)
        T8_list.append(T8)
        T8q_list.append(T8q)

    in_maps = []
    for k in range(N_CORES):
        sl = slice(k * C_CORE, (k + 1) * C_CORE)
        xwk = np.zeros((PAIRS, 128, XW_COLS), np.uint8)
        x8r = np.stack([x8[sl], r8[sl]], axis=-1)  # [32, H, WP, B, 2]
        xwk[:, :, X8R_OFF:W8_OFF] = (
            x8r.reshape(PAIRS, 128, WP * B * 2).view(np.uint8)
        )
        # weight blocks: [PAIRS, 128, N_MMS, 2, 128] -> DRSwInterleave layout
        wmk = np.zeros((PAIRS, 128, N_MMS, 2, 128), F8)
        for br in range(5):
            T8k = T8_list[br][sl]     # [32, ks, H, H]
            T8qk = T8q_list[br][sl]
            for mi, mm in enumerate(MM_PLAN[br]):
                m = _MM_BASE[br] + mi
                for si, (plane, dx, j, half) in enumerate(mm):
                    M = (T8qk if half else T8k)[:, j]     # [32, H, H] (hi, ho)
                    Mr = M.reshape(PAIRS, 2, H, H)
                    for ci in range(2):
                        blk = slice(ci * H, (ci + 1) * H)
                        wmk[:, blk, m, si, blk] = Mr[:, ci]
        # per partition row layout: [A127, B127, ..., A0, B0]
        wmk = wmk[:, :, :, :, ::-1]
        wmk = np.swapaxes(wmk, 3, 4)
        xwk[:, :, W8_OFF:] = wmk.reshape(PAIRS, 128, N_MMS * 2 * 128).view(np.uint8)
        in_maps.append(
            {"xw": np.ascontiguousarray(xwk), "scbi": scbi_all[k]}
        )
    return in_maps


def _assemble(results):
    y = np.empty((B, C, H, W), np.float32)
    for k in range(N_CORES):
        ytk = np.asarray(results[k]["yt"]).astype(np.float32)  # [PAIRS,128,B*W]
        ytk = ytk.reshape(PAIRS, 2, H, W, B).transpose(4, 0, 1, 2, 3)
        y[:, k * C_CORE : (k + 1) * C_CORE] = ytk.reshape(B, C_CORE, H, W)
    return y


def kernel_run(inputs, trace=False, tmpdir=None):
    if "nc" not in _CACHE:
        _CACHE["nc"] = build_nc()
    nc = _CACHE["nc"]
    in_maps = _host_prep(**inputs)
    res = run_bass_kernel_spmd(
        nc, in_maps, list(range(N_CORES)), trace=trace, tmpdir=tmpdir
    )
    return _assemble(res.results), res


def kernel(**inputs):
    out, _ = kernel_run(inputs, trace=False)
    return out


# revision 36
# speedup vs baseline: 1.0114x; 1.0114x over previous
"""Trainium2 Bass kernel for DilatedReparamConv (5-branch depthwise conv + BN + SiLU + identity BN).

out = BN_id(x) + sum_i silu(BN_i(dwconv_i(x)))   for branches
      (5,d1), (7,d2), (3,d3), (3,d4), (3,d5), all SAME padding.

Strategy (8 NeuronCores, SPMD):
  - Shard the 256 channels across 8 cores (32 ch/core, all 32 images).
  - Depthwise conv on TensorE in fp8 (e4m3) DoubleRowSwInterleave: per
    channel-pair, a block-diagonal banded Toeplitz matrix contracts the kh
    tap-stack in one matmul; kw taps are free-dim shifts into a W-padded x
    tile. DR mode runs 2 slots/cycle (0.5 cyc/col): the slot pair carries
    either (x8, resid) error feedback [EF] or two packed taps (x8@dxA,
    x8@dxB) [PACK], per the MM_PLAN below.
  - BN+SiLU fused into ScalarE PSUM->SBUF eviction (per-partition scale/bias
    APs), one [128, 2048] activation per branch (4 PSUM banks).
  - fp16 branch tiles; DVE tree-sum in fp16 (2x_1p mode); identity branch
    fused via affine_then_add custom DVE op. fp16 output.
"""

import sys

sys.path.insert(0, "/opt/trn_rl_repo")

import numpy as np
import ml_dtypes

import concourse.bass as bass
import concourse.mybir as mybir
from concourse import bacc, tile
from concourse.bass_utils import run_bass_kernel_spmd

# ---------------------------------------------------------------- problem dims
B, C, H, W = 32, 256, 64, 64
EPS = 1e-5
BRANCH_CFG = [(5, 1), (7, 2), (3, 3), (3, 4), (3, 5)]  # (kernel, dilation)
N_CORES = 8
C_CORE = C // N_CORES          # 32 channels per core
PAIRS = C_CORE // 2            # 16 channel-pairs per core
PAD = 6                        # max dilation*(ks-1)//2 across branches
WP = W + 2 * PAD               # padded width = 76

F8 = ml_dtypes.float8_e4m3
F16 = np.float16

W_CHUNK, N_CHUNKS = 16, 4      # 16 w-cols * B = 512 psum cols per chunk

# Which kw taps of each branch get error feedback (the rest are packed
# two-taps-per-matmul). "all" = full EF.
# P13-ctr: EF only the center kw tap of each branch (13 MMs total).
EF_SETS = [{2}, {3}, {1}, {1}, {1}]


def _branch_taps(br):
    ks, dil = BRANCH_CFG[br]
    pad = dil * (ks - 1) // 2
    return [(j, dil * j - pad) for j in range(ks)]


def _mm_plan():
    """Per branch: list of MMs. Each MM = (slotA, slotB), slot =
    (plane, dx, tap_j, half) with plane in {0:x8, 1:resid}; half -> use
    T8/2 matrix. Slots are ordered so that byte offset(A) < offset(B)."""
    plan = []
    for br in range(5):
        taps = _branch_taps(br)
        ef = EF_SETS[br]
        if ef is None:
            ef = {j for j, _ in taps}
        mms = []
        packed = [t for t in taps if t[0] not in ef]
        for j, dx in taps:
            if j in ef:
                # (x8@dx, resid@dx): offsets x8 < resid plane -> ordered
                mms.append(((0, dx, j, False), (1, dx, j, True)))
        for i in range(0, len(packed) - 1, 2):
            (ja, dxa), (jb, dxb) = packed[i], packed[i + 1]
            mms.append(((0, dxa, ja, False), (0, dxb, jb, False)))
        if len(packed) % 2:
            j, dx = packed[-1]
            # lone tap: EF it (resid slot is free anyway)
            mms.append(((0, dx, j, False), (1, dx, j, True)))
        plan.append(mms)
    return plan


MM_PLAN = _mm_plan()
N_MMS = sum(len(m) for m in MM_PLAN)

# packed per-partition byte layout of the xw tile (all uint8 columns)
# x8/resid are element-interleaved ([WP*B, 2] pairs); the identity branch is
# reconstructed on DVE as s*(x8 + resid/2) + b, so no natural-x plane.
X8R_OFF = 0                    # (x8, resid) pairs [WP*B, 2] fp8
W8_OFF = 2 * WP * B            # weights [N_MMS, 2, 128] fp8 interleaved
XW_COLS = W8_OFF + N_MMS * 2 * 128

_CACHE: dict = {}


# =====================================================================
# device build
# =====================================================================
def build_nc():
    nc = bacc.Bacc("TRN2", target_bir_lowering=False, debug=False, num_devices=N_CORES)
    f32 = mybir.dt.float32
    f16 = mybir.dt.float16
    u8 = mybir.dt.uint8
    f8 = mybir.dt.float8e4

    xw = nc.dram_tensor("xw", [PAIRS, 128, XW_COLS], u8, kind="ExternalInput").ap()
    scbi = nc.dram_tensor("scbi", [128, 2 * PAIRS * 8], f32, kind="ExternalInput").ap()
    yt = nc.dram_tensor("yt", [PAIRS, 128, B * W], f16, kind="ExternalOutput").ap()

    NBW = B * W  # 2048

    with tile.TileContext(nc) as tc:
        with (
            tc.tile_pool(name="consts", bufs=1) as consts,
            tc.tile_pool(name="xwp", bufs=6) as xwp,
            tc.tile_pool(name="accp", bufs=4) as accp,
            tc.tile_pool(name="tp", bufs=17) as tp,
            tc.tile_pool(name="tsum", bufs=6) as tsum,
            tc.tile_pool(name="psum", bufs=2, space="PSUM") as psum,
        ):
            scbi_t = consts.tile([128, 2 * PAIRS * 8], f32)
            nc.sync.dma_start(out=scbi_t[:], in_=scbi)
            sc_t = scbi_t[:, : PAIRS * 8]
            bi_t = scbi_t[:, PAIRS * 8 :]

            wf8 = scbi_t.bitcast(f8)

            def load_pair(p):
                xw_t = xwp.tile([128, XW_COLS], u8)
                # weights first (first matmul needs them), then the plane in
                # two pieces so early chunks can start before the tail lands
                nc.sync.dma_start(out=xw_t[:, W8_OFF:], in_=xw[p][:, W8_OFF:])
                q = W8_OFF // 2
                for i in range(2):
                    nc.sync.dma_start(
                        out=xw_t[:, i * q : (i + 1) * q],
                        in_=xw[p][:, i * q : (i + 1) * q],
                    )
                return xw_t

            def mm_branch(xw_t, p, br, warm=False):
                xw_f8 = xw_t.bitcast(f8)

                def rhs_ap(mm, cch):
                    # byte offset of slot s in the interleaved (x8, resid)
                    # plane: 2*element + plane
                    (pa, dxa, _, _), (pb, dxb, _, _) = mm
                    offa = 2 * (cch * W_CHUNK + PAD + dxa) * B + pa
                    offb = 2 * (cch * W_CHUNK + PAD + dxb) * B + pb
                    assert offb > offa
                    r = xw_f8[:, offa : offa + 2 * W_CHUNK * B].unsqueeze(1)
                    ap = r.ap
                    ap[1] = [offb - offa, 2]   # slot dim
                    ap[2] = [2, W_CHUNK * B]   # (w, b) columns, every other byte
                    r.ap = ap
                    return r

                mms = MM_PLAN[br]
                ps = psum.tile([128, NBW], f32)
                if warm:
                    # warm up the PE clock (HAM) during the initial DMA with
                    # dummy matmuls on the tiny already-loaded scbi bytes;
                    # the first real start=True matmul clears the garbage
                    for _ in range(16):
                        nc.tensor.matmul(
                            ps[:, :256],
                            wf8[:, :256],
                            wf8[:, :512].rearrange("p (n two) -> p two n", two=2),
                            start=True,
                            stop=True,
                            perf_mode=mybir.MatmulPerfMode.DoubleRowSwInterleave,
                        )
                # mi-outer: 4 consecutive matmuls share one weight load
                for mi, mm in enumerate(mms):
                    lhsT = xw_f8[
                        :,
                        W8_OFF + (_MM_BASE[br] + mi) * 256 :
                        W8_OFF + (_MM_BASE[br] + mi + 1) * 256,
                    ]
                    for cch in range(N_CHUNKS):
                        nc.tensor.matmul(
                            ps[:, cch * 512 : (cch + 1) * 512],
                            lhsT,
                            rhs_ap(mm, cch),
                            start=(mi == 0),
                            stop=(mi == len(mms) - 1),
                            perf_mode=mybir.MatmulPerfMode.DoubleRowSwInterleave,
                        )
                t = tp.tile([128, NBW], f16)
                halves = 1
                for h in range(halves):
                    lo = h * NBW // halves
                    hi = (h + 1) * NBW // halves
                    nc.scalar.activation(
                        t[:, lo:hi],
                        ps[:, lo:hi],
                        mybir.ActivationFunctionType.Silu,
                        bias=bi_t[:, p * 8 + br : p * 8 + br + 1],
                        scale=sc_t[:, p * 8 + br : p * 8 + br + 1],
                    )
                return t

            def finish_pair(xw_t, p, ts):
                xw_f8 = xw_t.bitcast(f8)

                def id_view(plane):
                    # x8 (plane 0) or resid (plane 1) at dx=0, [128, 2048]
                    s = 2 * PAD * B + plane
                    r = xw_f8[:, s : s + 2 * W * B]
                    ap = r.ap
                    ap[1] = [2, W * B]
                    r.ap = ap
                    return r

                # acc = s_id*(x8 + resid/2) + b_id + sum(t); tree-sum
                acc = accp.tile([128, NBW], f16)
                nc.vector.affine_then_add(
                    acc[:],
                    id_view(0),
                    ts[0][:],
                    sc_t[:, p * 8 + 5 : p * 8 + 6],
                    bi_t[:, p * 8 + 5 : p * 8 + 6],
                )
                nc.vector.affine_then_add(
                    acc[:],
                    id_view(1),
                    acc[:],
                    sc_t[:, p * 8 + 6 : p * 8 + 7],
                    bi_t[:, p * 8 + 6 : p * 8 + 7],
                )
                # serial adds: only one DVE op remains after the last eviction
                for i in range(1, 5):
                    nc.vector.tensor_tensor(
                        acc[:], acc[:], ts[i][:], op=mybir.AluOpType.add
                    )

                nc.sync.dma_start(out=yt[p][:, : NBW // 2], in_=acc[:, : NBW // 2])
                nc.sync.dma_start(out=yt[p][:, NBW // 2 :], in_=acc[:, NBW // 2 :])

            # three pairs in flight: each psum-tile reuse has two other
            # pairs' matmul blocks to cover the eviction latency
            groups = [[0, 1, 2], [3, 4, 5], [6, 7, 8], [9, 10, 11],
                      [12, 13], [14, 15]]
            for gi, g in enumerate(groups):
                if gi == 0:
                    # stage the very first pair's load alone so its DMA
                    # descriptors sit first in every engine ring
                    xs = [load_pair(g[0])]
                    tss = [[mm_branch(xs[0], g[0], 0, warm=True)]]
                    xs += [load_pair(p) for p in g[1:]]
                    tss += [[] for _ in g[1:]]
                    for br in range(5):
                        for k, p in enumerate(g):
                            if br == 0 and k == 0:
                                continue
                            tss[k].append(mm_branch(xs[k], p, br))
                else:
                    xs = [load_pair(p) for p in g]
                    tss = [[] for _ in g]
                    for br in range(5):
                        for k, p in enumerate(g):
                            tss[k].append(mm_branch(xs[k], p, br))
                for k, p in enumerate(g):
                    finish_pair(xs[k], p, tss[k])

    nc.compile()
    return nc


_MM_BASE = [0]
for _m in MM_PLAN:
    _MM_BASE.append(_MM_BASE[-1] + len(_m))


# ------------------------------------------------------------------ host prep
def _toeplitz_taps():
    """T[c, br][j] = [H, H] banded matrix (hi, ho) for kw tap j of branch br,
    returned as one [C, TOT_TAPS, H, H] array with per-branch tap offsets."""
    tot = sum(ks for ks, _ in BRANCH_CFG)
    base = [0]
    for ks, _ in BRANCH_CFG:
        base.append(base[-1] + ks)
    return tot, base


def _host_prep(x, id_bn, w5, w7, w3a, w3b, w3c, bn_gamma, bn_beta, bn_mean, bn_var):
    x = np.asarray(x, np.float32)
    weights = [np.asarray(w, np.float32) for w in (w5, w7, w3a, w3b, w3c)]
    id_bn = np.asarray(id_bn, np.float32)
    bn_gamma = np.asarray(bn_gamma, np.float32)
    bn_beta = np.asarray(bn_beta, np.float32)
    bn_mean = np.asarray(bn_mean, np.float32)
    bn_var = np.asarray(bn_var, np.float32)

    # per-(branch, channel) gain so T8 uses the e4m3 range well
    gain = np.empty((5, C), np.float32)
    for br in range(5):
        wmax = np.abs(weights[br][:, 0]).max(axis=(1, 2))
        gain[br] = 8.0 / np.maximum(wmax, 1e-8)

    # scale/bias tables: branches (scale / gain), identity, identity/2, zero
    S = np.zeros((8, C), np.float32)
    Bv = np.zeros((8, C), np.float32)
    for i in range(5):
        s = bn_gamma[i] / np.sqrt(bn_var[i] + EPS)
        S[i] = s / gain[i]
        Bv[i] = bn_beta[i] - bn_mean[i] * s
    s = id_bn[0] / np.sqrt(id_bn[3] + EPS)
    S[5] = s
    Bv[5] = id_bn[1] - id_bn[2] * s
    S[6] = s / 2.0

    # scbi per core: [128, PAIRS*8] scale | [128, PAIRS*8] bias
    # partition layout (ci*H + h), column layout p*8 + i
    Sr = S.reshape(8, N_CORES, PAIRS, 2)     # [8, k, p, ci]
    Br = Bv.reshape(8, N_CORES, PAIRS, 2)
    scbi_all = np.empty((N_CORES, 128, 2 * PAIRS * 8), np.float32)
    for ci in range(2):
        blk = slice(ci * H, (ci + 1) * H)
        scbi_all[:, blk, : PAIRS * 8] = np.transpose(
            Sr[:, :, :, ci], (1, 2, 0)
        ).reshape(N_CORES, 1, PAIRS * 8)
        scbi_all[:, blk, PAIRS * 8 :] = np.transpose(
            Br[:, :, :, ci], (1, 2, 0)
        ).reshape(N_CORES, 1, PAIRS * 8)

    # x planes, w-major free layout: [C, H, WP, B]
    xt = np.transpose(x, (1, 2, 3, 0))                 # [C, H, W, B]
    xp = np.zeros((C, H, WP, B), np.float32)
    xp[:, :, PAD : PAD + W, :] = xt
    x8 = xp.astype(F8)
    r8 = (2.0 * (xp - x8.astype(np.float32))).astype(F8)

    # per-tap banded Toeplitz matrices, with gain folded in
    # T8[c, br, j][hi, ho], T8q = e4m3(T8/2)
    T8_list = []
    T8q_list = []
    for br, (ks, dil) in enumerate(BRANCH_CFG):
        pad = dil * (ks - 1) // 2
        wbr = weights[br][:, 0] * gain[br][:, None, None]   # [C, ks, ks]
        Tb = np.zeros((C, ks, H, H), np.float32)
        for kw in range(ks):
            for kh in range(ks):
                off = dil * kh - pad
                ho = np.arange(max(0, -off), min(H, H - off))
                Tb[:, kw, ho + off, ho] = wbr[:, kh, kw][:, None]
        T8 = Tb.astype(F8)
        T8q = (T8.astype(np.float32) / 2.0).astype(F8)
        T8_list.append(T8)
        T8q_list.append(T8q)

    in_maps = []
    for k in range(N_CORES):
        sl = slice(k * C_CORE, (k + 1) * C_CORE)
        xwk = np.zeros((PAIRS, 128, XW_COLS), np.uint8)
        x8r = np.stack([x8[sl], r8[sl]], axis=-1)  # [32, H, WP, B, 2]
        xwk[:, :, X8R_OFF:W8_OFF] = (
            x8r.reshape(PAIRS, 128, WP * B * 2).view(np.uint8)
        )
        # weight blocks: [PAIRS, 128, N_MMS, 2, 128] -> DRSwInterleave layout
        wmk = np.zeros((PAIRS, 128, N_MMS, 2, 128), F8)
        for br in range(5):
            T8k = T8_list[br][sl]     # [32, ks, H, H]
            T8qk = T8q_list[br][sl]
            for mi, mm in enumerate(MM_PLAN[br]):
                m = _MM_BASE[br] + mi
                for si, (plane, dx, j, half) in enumerate(mm):
                    M = (T8qk if half else T8k)[:, j]     # [32, H, H] (hi, ho)
                    Mr = M.reshape(PAIRS, 2, H, H)
                    for ci in range(2):
                        blk = slice(ci * H, (ci + 1) * H)
                        wmk[:, blk, m, si, blk] = Mr[:, ci]
        # per partition row layout: [A127, B127, ..., A0, B0]
        wmk = wmk[:, :, :, :, ::-1]
        wmk = np.swapaxes(wmk, 3, 4)
        xwk[:, :, W8_OFF:] = wmk.reshape(PAIRS, 128, N_MMS * 2 * 128).view(np.uint8)
        in_maps.append(
            {"xw": np.ascontiguousarray(xwk), "scbi": scbi_all[k]}
        )
    return in_maps


def _assemble(results):
    y = np.empty((B, C, H, W), np.float32)
    for k in range(N_CORES):
        ytk = np.asarray(results[k]["yt"]).astype(np.float32)  # [PAIRS,128,B*W]
        ytk = ytk.reshape(PAIRS, 2, H, W, B).transpose(4, 0, 1, 2, 3)
        y[:, k * C_CORE : (k + 1) * C_CORE] = ytk.reshape(B, C_CORE, H, W)
    return y


def kernel_run(inputs, trace=False, tmpdir=None):
    if "nc" not in _CACHE:
        _CACHE["nc"] = build_nc()
    nc = _CACHE["nc"]
    in_maps = _host_prep(**inputs)
    res = run_bass_kernel_spmd(
        nc, in_maps, list(range(N_CORES)), trace=trace, tmpdir=tmpdir
    )
    return _assemble(res.results), res


def kernel(**inputs):
    out, _ = kernel_run(inputs, trace=False)
    return out
